# revision 1
# baseline (speedup 1.0000x reference)
"""Trainium2 Bass kernel for nn_DebugBertLayer_87093346828840.

Key observation: the reference overwrites q/k/v with the constant 0.01, so
softmax(scores) is uniform and ctx == 0.01 everywhere.  Hence
    attn_out = LN1(hidden + cvec),   cvec = 0.01 * Wo.sum(axis=1) + bo
and the only real device work is the FFN:
    out = LN2( gelu(attn_out @ Wi.T + bi) @ Wf.T + bf + attn_out )

Sharding: pure data-parallel over the 8192 tokens -> 1024 tokens/core on 8
NeuronCores, no collectives.  Matmuls run in bf16 (fp32 PE matmul is 4x
slower), fp32 PSUM accumulation, fp32 LN/residual path.

Per-core dataflow (token tile = 128 tokens, two token-halves of 512):
  1. LN1 in token-major layout ((x+cvec), bn_stats over the 768 features);
     bf16 copy of the result on the Scalar engine.
  2. PE-transpose the bf16 LN output to feature-major aT [768, 512] per half.
  3. mm1 per half: hT[ff, tok] accumulated over 6 k-tiles (WiT stationary),
     gelu(+bi per-partition bias) psum -> bf16 hT in SBUF.  Phase A (tokens
     0:512) starts after only half the LN1 work; phase B's LN1/transposes
     hide under phase A's matmuls.
  4. mm2: y[tok, 768] accumulated over 24 ff-tiles with hT slices stationary,
     WfT streaming -> token-major psum.
  5. y + a (residual, fp32) + bf, LN2, DMA out.  No output-side transpose
     because mm2's stationary operand is the feature-major hT.
"""

import os
import sys

for _p in ("/opt/trn_rl_repo", "/root/.axon_site/_ro/trn_rl_repo"):
    if os.path.isdir(_p) and _p not in sys.path:
        sys.path.insert(0, _p)

import numpy as np
import ml_dtypes

import concourse.bass as bass
import concourse.bacc as bacc
import concourse.tile as tile
from concourse import mybir
from concourse.bass_utils import run_bass_kernel_spmd

F32 = mybir.dt.float32
BF16 = mybir.dt.bfloat16
AF = mybir.ActivationFunctionType
ALU = mybir.AluOpType
BF16NP = ml_dtypes.bfloat16

D = 768           # d_model
FF = 3072         # d_ff
NCORE = 8
TOK = 8192        # total tokens (4 x 2048)
TPC = TOK // NCORE  # 1024 tokens per core
KD = D // 128     # 6 k-tiles over d_model
MF = FF // 128    # 24 tiles over d_ff
NT = TPC // 128   # 8 token tiles per core
NTH = NT // 2     # token tiles per half
HALF = TPC // 2   # 512
LN_EPS = 1e-12

_NC_CACHE = {}
LAST_RESULTS = None
RUN_KWARGS = {}


def _ln_tile(nc, pstat, eps_t, s_t, gb, apply_gb):
    """In-place LayerNorm over the free dim (768) of s_t [128, 768] f32.

    rstd is computed on the Vector engine (bit-trick seed + 2 Newton steps)
    instead of ScalarE Sqrt: the ACT engine then only ever runs Gelu, which
    avoids ~1.3-7.6us activation-table reloads on every Sqrt<->Gelu switch.
    """
    g_b, b_b = gb
    sr = s_t.rearrange("p (n s) -> p n s", s=384)
    stats = pstat.tile([128, 2, 6], F32, tag="stats")
    for i in range(2):
        nc.vector.bn_stats(out=stats[:, i, :], in_=sr[:, i, :])
    mv = pstat.tile([128, 2], F32, tag="mv")
    nc.vector.bn_aggr(out=mv[:], in_=stats[:])
    v = mv[:, 1:2]
    # NOTE: var + LN_EPS (1e-12) == var in fp32 for any var > ~1e-5, which
    # always holds here (LN inputs are ~N(0,1)); skip the extra DVE pass
    rst = pstat.tile([128, 1], F32, tag="rst")
    nrt = pstat.tile([128, 1], F32, tag="nrt")
    # y0 = bitcast(0x5f3759df - (bits(v) >> 1)): ~3.4% rsqrt seed
    nc.vector.tensor_scalar(out=rst.bitcast(mybir.dt.int32)[:],
                            in0=v.bitcast(mybir.dt.int32),
                            scalar1=1, scalar2=None,
                            op0=ALU.logical_shift_right)
    nc.vector.tensor_scalar(out=rst.bitcast(mybir.dt.int32)[:],
                            in0=rst.bitcast(mybir.dt.int32)[:],
                            scalar1=-1, scalar2=0x5F3759DF,
                            op0=ALU.mult, op1=ALU.add)
    for _ in range(2):  # y <- y*(1.5 - 0.5*v*y^2); 2 steps -> ~1e-6 rel
        nc.vector.tensor_mul(out=nrt[:], in0=rst[:], in1=rst[:])
        nc.vector.tensor_mul(out=nrt[:], in0=nrt[:], in1=v)
        nc.vector.tensor_scalar(out=nrt[:], in0=nrt[:], scalar1=-0.5,
                                scalar2=1.5, op0=ALU.mult, op1=ALU.add)
        nc.vector.tensor_mul(out=rst[:], in0=rst[:], in1=nrt[:])
    nc.vector.tensor_scalar(out=s_t[:], in0=s_t[:], scalar1=mv[:, 0:1],
                            scalar2=rst[:], op0=ALU.subtract, op1=ALU.mult)
    if apply_gb:
        nc.vector.tensor_mul(out=s_t[:], in0=s_t[:], in1=g_b[:])
        nc.vector.tensor_add(out=s_t[:], in0=s_t[:], in1=b_b[:])


def _emit_body(nc, tc, pools, tensors, flags, x_pre=None, emit_wf=None):
    """Emit one full layer computation (one 'rep').

    x_pre: optionally pre-loaded x tiles (first rep: their DMAs were issued
    ahead of the weight DMAs so LN1 isn't stuck behind 9MB of weights).
    emit_wf: callback emitting the wf weight DMAs (first rep defers them
    until after phase A so they don't delay the aT transposes).
    """
    (pw, px, pbig, pabf, pstat, pout, ps1, psm) = pools
    (x, y, wi_tiles, wf_tiles, ident_sb, cvec_b, g1_b, b1_b, g2_b, b2_b,
     bfv_b, bi_sb, eps_t) = tensors

    a_tiles = [None] * NT
    # phases over token-tile ranges: a 512-token half (PE transpose in the
    # idle prologue) then a 512-token half (DMA transpose, hidden under mm1).
    # A 256-token quarter split simmed the same but measured ~10% worse on
    # hardware (extra ldweights traffic at N=256).
    PHASES = [(0, 4, False), (4, 8, True)]
    aT_ph = {}

    def ln1_and_transpose(ph):
        t0p, t1p, use_dma = PHASES[ph]
        width = (t1p - t0p) * 128
        aT = pbig.tile([128, KD * width], BF16, tag=f"aT{ph}")
        aT_ph[ph] = aT
        for tt, t in enumerate(range(t0p, t1p)):
            if x_pre is not None:
                x_t = x_pre[t]
            else:
                x_t = px.tile([128, D], F32, tag="xa")
                nc.sync.dma_start(out=x_t[:], in_=x[t * 128:(t + 1) * 128, :])
            nc.vector.tensor_add(out=x_t[:], in0=x_t[:], in1=cvec_b[:])
            _ln_tile(nc, pstat, eps_t, x_t, (g1_b, b1_b), flags["g1b1"])
            a_tiles[t] = x_t
            if not use_dma:
                # prologue: PE is idle anyway; fp32 PE-transpose straight from
                # the LN output (the psum->SBUF copy does the bf16 cast)
                for k in range(KD):
                    dst = aT[:, k * width + tt * 128: k * width + (tt + 1) * 128]
                    ps_tr = psm.tile([128, 128], F32, tag="psm")
                    nc.tensor.transpose(ps_tr[:], x_t[:, k * 128:(k + 1) * 128],
                                        ident_sb[:])
                    # ACT copy: keeps the psum drain off the DVE, whose
                    # LayerNorm ladder gates mm1 phase A
                    nc.scalar.activation(out=dst, in_=ps_tr[:], func=AF.Copy,
                                         scale=1.0)
            else:
                # steady state: PE is busy in mm1; use the DMA xbar transpose
                # (bf16 only, hence the cast)
                a_bf = pabf.tile([128, D], BF16, tag="abf")
                nc.vector.tensor_copy(out=a_bf[:], in_=x_t[:])
                for k in range(KD):
                    dst = aT[:, k * width + tt * 128: k * width + (tt + 1) * 128]
                    nc.sync.dma_start(out=dst, in_=a_bf[:, k * 128:(k + 1) * 128],
                                      transpose=True)

    def mm1_phase(ph, hT):
        t0p, t1p, _ = PHASES[ph]
        width = (t1p - t0p) * 128
        off = t0p * 128
        aT = aT_ph[ph]
        for m in range(MF):
            ps_a = ps1.tile([128, 512], F32, tag="hps")
            for k in range(KD):
                lhsT = wi_tiles[k][:, m * 128:(m + 1) * 128]
                nc.tensor.matmul(ps_a[:, 0:width], lhsT,
                                 aT[:, k * width:(k + 1) * width],
                                 start=(k == 0), stop=(k == KD - 1))
            if flags["bi"]:
                nc.scalar.activation(
                    out=hT[:, m * TPC + off: m * TPC + off + width],
                    in_=ps_a[:, 0:width], func=AF.Gelu, bias=bi_sb[:, m:m + 1],
                    scale=1.0)
            else:
                nc.scalar.activation(
                    out=hT[:, m * TPC + off: m * TPC + off + width],
                    in_=ps_a[:, 0:width], func=AF.Gelu, scale=1.0)

    # ---- LN1+transpose / mm1, software-pipelined across the three phases ----
    hT = pbig.tile([128, MF * TPC], BF16, tag="hT")   # [ff-part, m*1024 + tok]
    ln1_and_transpose(0)
    mm1_phase(0, hT)
    if emit_wf is not None:
        emit_wf(0, MF)   # mm2 weights: queue them only after phase A's DMAs
    ln1_and_transpose(1)
    mm1_phase(1, hT)

    # ---------------- mm2 + residual + LN2 ----------------
    for t in range(NT):
        ps2 = psm.tile([128, D], F32, tag="psm")
        for m in range(MF):
            lhsT = hT[:, m * TPC + t * 128: m * TPC + (t + 1) * 128]
            nc.tensor.matmul(ps2[:, 0:512], lhsT, wf_tiles[m][:, 0:512],
                             start=(m == 0), stop=(m == MF - 1))
            nc.tensor.matmul(ps2[:, 512:768], lhsT, wf_tiles[m][:, 512:768],
                             start=(m == 0), stop=(m == MF - 1))
        s_t = pout.tile([128, D], F32, tag="s")
        nc.vector.tensor_add(out=s_t[:], in0=ps2[:], in1=a_tiles[t][:])
        if flags["bfv"]:
            nc.vector.tensor_add(out=s_t[:], in0=s_t[:], in1=bfv_b[:])
        _ln_tile(nc, pstat, eps_t, s_t, (g2_b, b2_b), flags["g2b2"])
        nc.sync.dma_start(out=y[t * 128:(t + 1) * 128, :], in_=s_t[:])


def _bcast_ap(handle, n):
    """AP that broadcasts a [n]-vector across 128 partitions for DMA."""
    return bass.AP(tensor=handle, offset=0, ap=[[0, 128], [1, n]])


def _build(n_reps=1, flag_key=(True, True, True, True)):
    cache_key = (n_reps, flag_key)
    if cache_key in _NC_CACHE:
        return _NC_CACHE[cache_key]
    flags = dict(zip(("g1b1", "g2b2", "bi", "bfv"), flag_key))
    nc = bacc.Bacc("TRN2", target_bir_lowering=False, debug=False,
                   num_devices=NCORE)
    x = nc.dram_tensor("x", [TPC, D], F32, kind="ExternalInput")
    wi = nc.dram_tensor("wi", [KD, 128, FF], BF16, kind="ExternalInput")
    wf = nc.dram_tensor("wf", [MF, 128, D], BF16, kind="ExternalInput")
    cvec = nc.dram_tensor("cvec", [D], F32, kind="ExternalInput")
    g1 = nc.dram_tensor("g1", [D], F32, kind="ExternalInput")
    b1 = nc.dram_tensor("b1", [D], F32, kind="ExternalInput")
    g2 = nc.dram_tensor("g2", [D], F32, kind="ExternalInput")
    b2 = nc.dram_tensor("b2", [D], F32, kind="ExternalInput")
    bfv = nc.dram_tensor("bfv", [D], F32, kind="ExternalInput")
    bi = nc.dram_tensor("bi", [FF], F32, kind="ExternalInput")
    y = nc.dram_tensor("y", [TPC, D], F32, kind="ExternalOutput")
    ident = nc.inline_tensor(np.eye(128, dtype=np.float32), name="ident")

    with tile.TileContext(nc) as tc:
        with (
            tc.tile_pool(name="pw", bufs=1) as pw,
            tc.tile_pool(name="px", bufs=NT) as px,
            tc.tile_pool(name="pbig", bufs=1) as pbig,
            tc.tile_pool(name="pabf", bufs=4) as pabf,
            tc.tile_pool(name="pstat", bufs=4) as pstat,
            tc.tile_pool(name="pout", bufs=3) as pout,
            tc.tile_pool(name="ps1", bufs=3, space="PSUM") as ps1,
            tc.tile_pool(name="psm", bufs=2, space="PSUM") as psm,
        ):
            # x tiles first: LN1(t0) is the head of the critical chain
            x_pre = []
            for t in range(NTH):
                x_t = px.tile([128, D], F32, tag="xa")
                nc.sync.dma_start(out=x_t[:], in_=x[t * 128:(t + 1) * 128, :])
                x_pre.append(x_t)

            # broadcast constants go on the (parallel) SWDGE queue
            def bcast(handle, n, tag):
                t = pw.tile([128, n], F32, tag=tag)
                nc.gpsimd.dma_start(out=t[:], in_=_bcast_ap(handle, n))
                return t

            cvec_b = bcast(cvec, D, "cvec")
            g1_b = bcast(g1, D, "g1") if flags["g1b1"] else None
            b1_b = bcast(b1, D, "b1") if flags["g1b1"] else None
            g2_b = bcast(g2, D, "g2") if flags["g2b2"] else None
            b2_b = bcast(b2, D, "b2") if flags["g2b2"] else None
            bfv_b = bcast(bfv, D, "bfv") if flags["bfv"] else None
            bi_sb = None
            if flags["bi"]:
                # bi as [128, 24]: column m holds bi[m*128 : (m+1)*128]
                bi_sb = pw.tile([128, MF], F32, tag="bi")
                nc.gpsimd.dma_start(
                    out=bi_sb[:],
                    in_=bass.AP(tensor=bi, offset=0, ap=[[1, 128], [128, MF]]))
            eps_t = pw.tile([128, 1], F32, tag="eps")
            nc.vector.memset(eps_t[:], LN_EPS)
            # dummy Gelu at t=0: hoists the one ACT function-table load into
            # the DMA prologue where it is fully hidden
            nc.scalar.activation(out=eps_t[:], in_=eps_t[:], func=AF.Gelu,
                                 scale=1.0)
            nc.vector.memset(eps_t[:], LN_EPS)

            # DMA order = HBM arrival order: first-half x tiles, then the
            # mm1 weights, then second-half x, then the mm2 weights.
            ident_sb = pw.tile([128, 128], F32, tag="ident")
            nc.sync.dma_start(out=ident_sb[:], in_=ident.ap())
            wi_tiles = []
            for k in range(KD):
                wt = pw.tile([128, FF], BF16, tag=f"wi{k}")
                nc.sync.dma_start(out=wt[:], in_=wi[k])
                wi_tiles.append(wt)
            for t in range(NTH, NT):
                x_t = px.tile([128, D], F32, tag="xa")
                nc.sync.dma_start(out=x_t[:], in_=x[t * 128:(t + 1) * 128, :])
                x_pre.append(x_t)
            wf_tiles = []
            for m in range(MF):
                wt = pw.tile([128, D], BF16, tag=f"wf{m}")
                wf_tiles.append(wt)

            def emit_wf(m0, m1):
                for m in range(m0, m1):
                    nc.sync.dma_start(out=wf_tiles[m][:], in_=wf[m])

            tensors = (x, y, wi_tiles, wf_tiles, ident_sb, cvec_b, g1_b, b1_b,
                       g2_b, b2_b, bfv_b, bi_sb, eps_t)
            pools = (pw, px, pbig, pabf, pstat, pout, ps1, psm)
            if isinstance(n_reps, tuple):  # ("loop", n) -> dynamic Tile loop
                emit_wf(0, MF)
                with tc.For_i(0, n_reps[1], 1):
                    _emit_body(nc, tc, pools, tensors, flags)
            else:
                for i in range(n_reps):
                    _emit_body(nc, tc, pools, tensors, flags,
                               x_pre=x_pre if i == 0 else None,
                               emit_wf=emit_wf if i == 0 else None)

    nc.compile()
    _NC_CACHE[cache_key] = nc
    return nc


def _prep_inputs(hidden_states, Wo, bo, ln1_g, ln1_b, Wi, bi, Wf, bf,
                 ln2_g, ln2_b):
    x = np.ascontiguousarray(np.asarray(hidden_states, np.float32)
                             .reshape(TOK, D))
    Wo = np.asarray(Wo, np.float32)
    Wi = np.asarray(Wi, np.float32)
    Wf = np.asarray(Wf, np.float32)
    cvec = (0.01 * Wo.sum(axis=1) + np.asarray(bo, np.float32)).astype(np.float32)
    # wi layout: [k, p, f] = Wi.T[k*128+p, f]
    wi_prep = np.ascontiguousarray(
        Wi.T.reshape(KD, 128, FF).astype(BF16NP))
    # wf layout: [m, p, j] = Wf.T[m*128+p, j]
    wf_prep = np.ascontiguousarray(
        Wf.T.reshape(MF, 128, D).astype(BF16NP))
    common = {
        "wi": wi_prep, "wf": wf_prep, "cvec": cvec,
        "g1": np.asarray(ln1_g, np.float32), "b1": np.asarray(ln1_b, np.float32),
        "g2": np.asarray(ln2_g, np.float32), "b2": np.asarray(ln2_b, np.float32),
        "bfv": np.asarray(bf, np.float32), "bi": np.asarray(bi, np.float32),
    }
    in_maps = [dict(common, x=x[c * TPC:(c + 1) * TPC]) for c in range(NCORE)]
    flag_key = (
        not (np.all(ln1_g == 1.0) and np.all(ln1_b == 0.0)),
        not (np.all(ln2_g == 1.0) and np.all(ln2_b == 0.0)),
        bool(np.any(np.asarray(bi) != 0.0)),
        bool(np.any(np.asarray(bf) != 0.0)),
    )
    return in_maps, flag_key


def kernel(hidden_states, Wq, bq, Wk, bk, Wv, bv, Wo, bo, ln1_g, ln1_b,
           Wi, bi, Wf, bf, ln2_g, ln2_b):
    global LAST_RESULTS
    B, S, _ = hidden_states.shape
    in_maps, flag_key = _prep_inputs(hidden_states, Wo, bo, ln1_g, ln1_b,
                                     Wi, bi, Wf, bf, ln2_g, ln2_b)
    nc = _build(RUN_KWARGS.get("n_reps", 1), flag_key)
    res = run_bass_kernel_spmd(nc, in_maps, list(range(NCORE)),
                               **{k: v for k, v in RUN_KWARGS.items()
                                  if k != "n_reps"})
    LAST_RESULTS = res
    out = np.concatenate([res.results[c]["y"] for c in range(NCORE)], axis=0)
    return np.ascontiguousarray(out.reshape(B, S, D).astype(np.float32))



# revision 6
# speedup vs baseline: 1.5144x; 1.5144x over previous
"""Trainium2 Bass kernel for nn_DebugBertLayer_87093346828840.

Key observation: the reference overwrites q/k/v with the constant 0.01, so
softmax(scores) is uniform and ctx == 0.01 everywhere.  Hence
    attn_out = LN1(hidden + cvec),   cvec = 0.01 * Wo.sum(axis=1) + bo
and the only real device work is the FFN:
    out = LN2( gelu(attn_out @ Wi.T + bi) @ Wf.T + bf + attn_out )

Sharding: pure data-parallel over the 8192 tokens -> 1024 tokens/core on 8
NeuronCores, no collectives.

Matmuls run in fp8e4 (e4m3) with perf_mode=DoubleRow: 2 fp8 weights/PE cell,
2 MACs/cycle, contraction 256/instruction -> ~1.8x the bf16 PE throughput.
Scale plumbing (all folded into existing ops, zero extra passes):
  - weights quantized at x256 host-side (keeps |w|~0.02 out of fp8 subnormals)
  - LN1's rsqrt is scaled by 256, so a_tiles hold 256*attn_out (fp32)
  - the fp8 cast of a_tiles uses scale 1/8 -> activations at x32
  - mm1 psum = 8192*h_pre; ACT Gelu drains it with scale=1/8192 -> h exact,
    written straight to fp8 (scale 1)
  - mm2 psum = 256*(h@Wf.T); the residual add uses the pre-scaled a_tiles and
    LayerNorm is scale-invariant, so LN2 absorbs the x256 for free.
DoubleRow pairing: contraction index (p, i) -> feature 2p+i (adjacent pairs),
so a [128,128]-uint16 transpose of packed fp8 pairs produces exactly the
[p][i=2][tok] moving-operand layout mm1 needs.

Per-core dataflow (token tile = 128 tokens, two token-halves of 512):
  1. LN1 in token-major layout; fp8 cast (x 1/8) on the Vector engine.
  2. Transpose packed-pair uint16 view to feature-pair-major aT8: phase A via
     PE transpose (idle prologue), phase B via DMA xbar (hidden under mm1).
  3. mm1 per half: hT[ff, tok] accumulated over 3 DoubleRow k-tiles (256
     features each), Gelu(+bi, scale 1/8192) psum -> fp8 hT in SBUF.
  4. mm2: y[tok, 768] accumulated over 12 DoubleRow ff-tiles with hT slices
     stationary, wf8 streaming -> token-major psum.
  5. psum + a_tiles (residual, fp32) [+ 256*bf], LN2, DMA out.
"""

import os
import sys

for _p in ("/opt/trn_rl_repo", "/root/.axon_site/_ro/trn_rl_repo"):
    if os.path.isdir(_p) and _p not in sys.path:
        sys.path.insert(0, _p)

import numpy as np
import ml_dtypes

import concourse.bass as bass
import concourse.bacc as bacc
import concourse.tile as tile
from concourse import mybir
from concourse.bass_utils import run_bass_kernel_spmd

F32 = mybir.dt.float32
BF16 = mybir.dt.bfloat16
F8 = mybir.dt.float8e4
U16 = mybir.dt.uint16
AF = mybir.ActivationFunctionType
ALU = mybir.AluOpType
DR = mybir.MatmulPerfMode.DoubleRow
F8NP = mybir.dt.np(mybir.dt.float8e4)   # ml_dtypes.float8_e4m3

D = 768           # d_model
FF = 3072         # d_ff
NCORE = 8
TOK = 8192        # total tokens (4 x 2048)
TPC = TOK // NCORE  # 1024 tokens per core
KK = D // 256     # 3 DoubleRow k-tiles over d_model
MF = FF // 128    # 24 tiles over d_ff
JJ = FF // 256    # 12 DoubleRow ff-tiles
NT = TPC // 128   # 8 token tiles per core
NTH = NT // 2     # token tiles per half
HALF = TPC // 2   # 512
LN_EPS = 1e-12

S_W = 256.0       # weight quant scale
S_A = 32.0        # activation quant scale (cast scale = S_A / S_W = 1/8)

_NC_CACHE = {}
LAST_RESULTS = None
RUN_KWARGS = {}


def _ln_tile(nc, pstat, eps_t, s_t, gb, apply_gb, post_scale=1.0):
    """In-place LayerNorm over the free dim (768) of s_t [128, 768] f32.

    rstd is computed on the Vector engine (bit-trick seed + 2 Newton steps)
    instead of ScalarE Sqrt: the ACT engine then only ever runs Gelu, which
    avoids ~1.3-7.6us activation-table reloads on every Sqrt<->Gelu switch.
    post_scale is folded into rstd, so the output is post_scale*LN(x).
    """
    g_b, b_b = gb
    sr = s_t.rearrange("p (n s) -> p n s", s=384)
    stats = pstat.tile([128, 2, 6], F32, tag="stats")
    for i in range(2):
        nc.vector.bn_stats(out=stats[:, i, :], in_=sr[:, i, :])
    mv = pstat.tile([128, 2], F32, tag="mv")
    nc.vector.bn_aggr(out=mv[:], in_=stats[:])
    v = mv[:, 1:2]
    # NOTE: var + LN_EPS (1e-12) == var in fp32 for any var > ~1e-5, which
    # always holds here (LN inputs are ~N(0,1)); skip the extra DVE pass
    rst = pstat.tile([128, 1], F32, tag="rst")
    nrt = pstat.tile([128, 1], F32, tag="nrt")
    # y0 = bitcast(0x5f3759df - (bits(v) >> 1)): ~3.4% rsqrt seed
    nc.vector.tensor_scalar(out=rst.bitcast(mybir.dt.int32)[:],
                            in0=v.bitcast(mybir.dt.int32),
                            scalar1=1, scalar2=None,
                            op0=ALU.logical_shift_right)
    nc.vector.tensor_scalar(out=rst.bitcast(mybir.dt.int32)[:],
                            in0=rst.bitcast(mybir.dt.int32)[:],
                            scalar1=-1, scalar2=0x5F3759DF,
                            op0=ALU.mult, op1=ALU.add)
    for it in range(2):  # y <- y*(1.5 - 0.5*v*y^2); 2 steps -> ~1e-6 rel
        nc.vector.tensor_mul(out=nrt[:], in0=rst[:], in1=rst[:])
        nc.vector.tensor_mul(out=nrt[:], in0=nrt[:], in1=v)
        nc.vector.tensor_scalar(out=nrt[:], in0=nrt[:], scalar1=-0.5,
                                scalar2=1.5, op0=ALU.mult, op1=ALU.add)
        if it == 1 and post_scale != 1.0:
            # fold post_scale into the last Newton step's nrt factor
            nc.vector.tensor_scalar(out=nrt[:], in0=nrt[:],
                                    scalar1=post_scale, scalar2=None,
                                    op0=ALU.mult)
        nc.vector.tensor_mul(out=rst[:], in0=rst[:], in1=nrt[:])
    nc.vector.tensor_scalar(out=s_t[:], in0=s_t[:], scalar1=mv[:, 0:1],
                            scalar2=rst[:], op0=ALU.subtract, op1=ALU.mult)
    if apply_gb:
        nc.vector.tensor_mul(out=s_t[:], in0=s_t[:], in1=g_b[:])
        nc.vector.tensor_add(out=s_t[:], in0=s_t[:], in1=b_b[:])


def _emit_body(nc, tc, pools, tensors, flags, x_pre=None, emit_wf=None):
    """Emit one full layer computation (one 'rep')."""
    (pw, px, pbig, pabf, pstat, pout, ps1, psm, ptr) = pools
    (x, y, wi8_sb, wf8_tiles, ident_sb, cvec_b, g1_b, b1_b, g2_b, b2_b,
     bfv_b, bi_sb, eps_t) = tensors

    wi8_r = wi8_sb.rearrange("p (c i f) -> p c i f", c=KK, i=2)

    a_tiles = [None] * NT
    # phases over token-tile ranges: a 512-token half (PE transpose in the
    # idle prologue) then a 512-token half (DMA transpose, hidden under mm1).
    PHASES = [(0, 4, True), (4, 8, True)]
    aT_ph = {}

    def ln1_and_transpose(ph):
        t0p, t1p, use_dma = PHASES[ph]
        width = (t1p - t0p) * 128
        # fp8 tile laid out [p][c][tok][i]: u16 view column c*width+t packs
        # the adjacent-feature pair (2p, 2p+1) of double-k-tile c at token t
        aT8 = pbig.tile([128, KK * width * 2], F8, tag=f"aT{ph}")
        aT8_u16 = aT8.bitcast(U16)  # [128, KK*width]
        aT_ph[ph] = aT8
        for tt, t in enumerate(range(t0p, t1p)):
            if x_pre is not None:
                x_t = x_pre[t]
            else:
                x_t = px.tile([128, D], F32, tag="xa")
                nc.sync.dma_start(out=x_t[:], in_=x[t * 128:(t + 1) * 128, :])
            nc.vector.tensor_add(out=x_t[:], in0=x_t[:], in1=cvec_b[:])
            _ln_tile(nc, pstat, eps_t, x_t, (g1_b, b1_b), flags["g1b1"],
                     post_scale=S_W)
            a_tiles[t] = x_t
            a8 = pabf.tile([128, D], F8, tag="a8")
            nc.vector.tensor_scalar(out=a8[:], in0=x_t[:],
                                    scalar1=S_A / S_W, scalar2=None,
                                    op0=ALU.mult)
            a8u = a8.bitcast(U16)  # [128, 384] feature pairs
            for c in range(KK):
                dst = aT8_u16[:, c * width + tt * 128:
                              c * width + (tt + 1) * 128]
                if not use_dma:
                    # prologue: PE is idle anyway; transpose the u16 pair view
                    ps_tr = ptr.tile([128, 128], U16, tag="ptr")
                    nc.tensor.transpose(ps_tr[:],
                                        a8u[:, c * 128:(c + 1) * 128],
                                        ident_sb[:])
                    nc.vector.tensor_copy(out=dst, in_=ps_tr[:])
                else:
                    # steady state: PE is busy in mm1; use the DMA xbar
                    nc.sync.dma_start(out=dst,
                                      in_=a8u[:, c * 128:(c + 1) * 128],
                                      transpose=True)

    def mm1_phase(ph, hT):
        t0p, t1p, _ = PHASES[ph]
        width = (t1p - t0p) * 128
        off = t0p * 128
        aT8 = aT_ph[ph]
        # moving operand [p][i=2][tok]: i stride 1 (packed pair), tok stride 2
        aT_r = aT8.rearrange("p (c t i) -> p c i t", c=KK, i=2)
        for m in range(MF):
            ps_a = ps1.tile([128, 512], F32, tag="hps")
            for c in range(KK):
                lhsT = wi8_r[:, c, :, m * 128:(m + 1) * 128]
                nc.tensor.matmul(ps_a[:, 0:width], lhsT,
                                 aT_r[:, c, :, 0:width],
                                 start=(c == 0), stop=(c == KK - 1),
                                 perf_mode=DR)
            if flags["bi"]:
                nc.scalar.activation(
                    out=hT[:, m * TPC + off: m * TPC + off + width],
                    in_=ps_a[:, 0:width], func=AF.Gelu, bias=bi_sb[:, m:m + 1],
                    scale=1.0 / (S_A * S_W))
            else:
                nc.scalar.activation(
                    out=hT[:, m * TPC + off: m * TPC + off + width],
                    in_=ps_a[:, 0:width], func=AF.Gelu,
                    scale=1.0 / (S_A * S_W))

    # ---- LN1+transpose / mm1, software-pipelined across the phases ----
    hT = pbig.tile([128, MF * TPC], F8, tag="hT")   # [ff-part, m*1024 + tok]
    ln1_and_transpose(0)
    mm1_phase(0, hT)
    if emit_wf is not None:
        emit_wf(0, JJ)   # mm2 weights: queue them only after phase A's DMAs
    ln1_and_transpose(1)
    mm1_phase(1, hT)

    # ---------------- mm2 + residual + LN2 ----------------
    # stationary hT slice [p][i=2][tok]: ff = (2*jj + i)*128 + p
    hT_r = hT.rearrange("p (j i t) -> p j i t", j=JJ, i=2)
    for t in range(NT):
        ps2 = psm.tile([128, D], F32, tag="psm")
        for jj in range(JJ):
            lhsT = hT_r[:, jj, :, t * 128:(t + 1) * 128]
            wf_r = wf8_tiles[jj].rearrange("p (i d) -> p i d", i=2)
            nc.tensor.matmul(ps2[:, 0:512], lhsT, wf_r[:, :, 0:512],
                             start=(jj == 0), stop=(jj == JJ - 1),
                             perf_mode=DR)
            nc.tensor.matmul(ps2[:, 512:768], lhsT, wf_r[:, :, 512:768],
                             start=(jj == 0), stop=(jj == JJ - 1),
                             perf_mode=DR)
        s_t = pout.tile([128, D], F32, tag="s")
        nc.vector.tensor_add(out=s_t[:], in0=ps2[:], in1=a_tiles[t][:])
        if flags["bfv"]:
            nc.vector.tensor_add(out=s_t[:], in0=s_t[:], in1=bfv_b[:])
        _ln_tile(nc, pstat, eps_t, s_t, (g2_b, b2_b), flags["g2b2"])
        nc.sync.dma_start(out=y[t * 128:(t + 1) * 128, :], in_=s_t[:])


def _bcast_ap(handle, n):
    """AP that broadcasts a [n]-vector across 128 partitions for DMA."""
    return bass.AP(tensor=handle, offset=0, ap=[[0, 128], [1, n]])


def _build(n_reps=1, flag_key=(True, True, True, True)):
    cache_key = (n_reps, flag_key)
    if cache_key in _NC_CACHE:
        return _NC_CACHE[cache_key]
    flags = dict(zip(("g1b1", "g2b2", "bi", "bfv"), flag_key))
    nc = bacc.Bacc("TRN2", target_bir_lowering=False, debug=False,
                   num_devices=NCORE)
    x = nc.dram_tensor("x", [TPC, D], F32, kind="ExternalInput")
    wi = nc.dram_tensor("wi", [128, KK * 2 * FF], F8, kind="ExternalInput")
    wf = nc.dram_tensor("wf", [JJ, 128, 2 * D], F8, kind="ExternalInput")
    cvec = nc.dram_tensor("cvec", [D], F32, kind="ExternalInput")
    g1 = nc.dram_tensor("g1", [D], F32, kind="ExternalInput")
    b1 = nc.dram_tensor("b1", [D], F32, kind="ExternalInput")
    g2 = nc.dram_tensor("g2", [D], F32, kind="ExternalInput")
    b2 = nc.dram_tensor("b2", [D], F32, kind="ExternalInput")
    bfv = nc.dram_tensor("bfv", [D], F32, kind="ExternalInput")
    bi = nc.dram_tensor("bi", [FF], F32, kind="ExternalInput")
    y = nc.dram_tensor("y", [TPC, D], F32, kind="ExternalOutput")
    ident = nc.inline_tensor(np.eye(128, dtype=np.uint16), name="ident")

    with tile.TileContext(nc) as tc:
        with (
            tc.tile_pool(name="pw", bufs=1) as pw,
            tc.tile_pool(name="px", bufs=NT) as px,
            tc.tile_pool(name="pbig", bufs=1) as pbig,
            tc.tile_pool(name="pabf", bufs=4) as pabf,
            tc.tile_pool(name="pstat", bufs=4) as pstat,
            tc.tile_pool(name="pout", bufs=3) as pout,
            tc.tile_pool(name="ps1", bufs=2, space="PSUM") as ps1,
            tc.tile_pool(name="psm", bufs=2, space="PSUM") as psm,
            tc.tile_pool(name="ptr", bufs=2, space="PSUM") as ptr,
        ):
            # x tiles first: LN1(t0) is the head of the critical chain
            x_pre = []
            for t in range(NTH):
                x_t = px.tile([128, D], F32, tag="xa")
                nc.sync.dma_start(out=x_t[:], in_=x[t * 128:(t + 1) * 128, :])
                x_pre.append(x_t)

            # broadcast constants go on the (parallel) SWDGE queue
            def bcast(handle, n, tag):
                t = pw.tile([128, n], F32, tag=tag)
                nc.gpsimd.dma_start(out=t[:], in_=_bcast_ap(handle, n))
                return t

            cvec_b = bcast(cvec, D, "cvec")
            g1_b = bcast(g1, D, "g1") if flags["g1b1"] else None
            b1_b = bcast(b1, D, "b1") if flags["g1b1"] else None
            g2_b = bcast(g2, D, "g2") if flags["g2b2"] else None
            b2_b = bcast(b2, D, "b2") if flags["g2b2"] else None
            bfv_b = bcast(bfv, D, "bfv") if flags["bfv"] else None
            bi_sb = None
            if flags["bi"]:
                # bi as [128, 24]: column m holds bi[m*128 : (m+1)*128]
                bi_sb = pw.tile([128, MF], F32, tag="bi")
                nc.gpsimd.dma_start(
                    out=bi_sb[:],
                    in_=bass.AP(tensor=bi, offset=0, ap=[[1, 128], [128, MF]]))
            eps_t = pw.tile([128, 1], F32, tag="eps")
            nc.vector.memset(eps_t[:], LN_EPS)
            # dummy Gelu at t=0: hoists the one ACT function-table load into
            # the DMA prologue where it is fully hidden
            nc.scalar.activation(out=eps_t[:], in_=eps_t[:], func=AF.Gelu,
                                 scale=1.0)
            nc.vector.memset(eps_t[:], LN_EPS)

            # DMA order = HBM arrival order: first-half x tiles, then the
            # mm1 weights, then second-half x, then the mm2 weights.
            ident_sb = pw.tile([128, 128], U16, tag="ident")
            nc.sync.dma_start(out=ident_sb[:], in_=ident.ap())
            wi8_sb = pw.tile([128, KK * 2 * FF], F8, tag="wi8")
            nc.sync.dma_start(out=wi8_sb[:], in_=wi[:, :])
            for t in range(NTH, NT):
                x_t = px.tile([128, D], F32, tag="xa")
                nc.sync.dma_start(out=x_t[:], in_=x[t * 128:(t + 1) * 128, :])
                x_pre.append(x_t)
            wf8_tiles = []
            for jj in range(JJ):
                wt = pw.tile([128, 2 * D], F8, tag=f"wf{jj}")
                wf8_tiles.append(wt)

            def emit_wf(j0, j1):
                for jj in range(j0, j1):
                    nc.sync.dma_start(out=wf8_tiles[jj][:], in_=wf[jj])

            tensors = (x, y, wi8_sb, wf8_tiles, ident_sb, cvec_b, g1_b, b1_b,
                       g2_b, b2_b, bfv_b, bi_sb, eps_t)
            pools = (pw, px, pbig, pabf, pstat, pout, ps1, psm, ptr)
            if isinstance(n_reps, tuple):  # ("loop", n) -> dynamic Tile loop
                emit_wf(0, JJ)
                with tc.For_i(0, n_reps[1], 1):
                    _emit_body(nc, tc, pools, tensors, flags)
            else:
                for i in range(n_reps):
                    _emit_body(nc, tc, pools, tensors, flags,
                               x_pre=x_pre if i == 0 else None,
                               emit_wf=emit_wf if i == 0 else None)

    nc.compile()
    _NC_CACHE[cache_key] = nc
    return nc


def _prep_inputs(hidden_states, Wo, bo, ln1_g, ln1_b, Wi, bi, Wf, bf,
                 ln2_g, ln2_b):
    x = np.ascontiguousarray(np.asarray(hidden_states, np.float32)
                             .reshape(TOK, D))
    Wo = np.asarray(Wo, np.float32)
    Wi = np.asarray(Wi, np.float32)
    Wf = np.asarray(Wf, np.float32)
    cvec = (0.01 * Wo.sum(axis=1) + np.asarray(bo, np.float32)).astype(np.float32)
    # wi layout [p, kk, i, f] = Wi.T[kk*256 + 2p + i, f] * S_W  (fp8)
    wi_s = (Wi.T * S_W).astype(np.float32)          # [D, FF]
    wi_prep = np.ascontiguousarray(
        wi_s.reshape(KK, 128, 2, FF).transpose(1, 0, 2, 3)
        .reshape(128, KK * 2 * FF).astype(F8NP))
    # wf layout [jj, p, i, d] = Wf.T[(2jj + i)*128 + p, d] * S_W  (fp8)
    wf_s = (Wf.T * S_W).astype(np.float32)          # [FF, D]
    wf_prep = np.ascontiguousarray(
        wf_s.reshape(JJ, 2, 128, D).transpose(0, 2, 1, 3)
        .reshape(JJ, 128, 2 * D).astype(F8NP))
    common = {
        "wi": wi_prep, "wf": wf_prep, "cvec": cvec,
        "g1": np.asarray(ln1_g, np.float32),
        "b1": np.asarray(ln1_b, np.float32) * np.float32(S_W),
        "g2": np.asarray(ln2_g, np.float32), "b2": np.asarray(ln2_b, np.float32),
        "bfv": np.asarray(bf, np.float32) * np.float32(S_W),
        "bi": np.asarray(bi, np.float32),
    }
    in_maps = [dict(common, x=x[c * TPC:(c + 1) * TPC]) for c in range(NCORE)]
    flag_key = (
        not (np.all(ln1_g == 1.0) and np.all(ln1_b == 0.0)),
        not (np.all(ln2_g == 1.0) and np.all(ln2_b == 0.0)),
        bool(np.any(np.asarray(bi) != 0.0)),
        bool(np.any(np.asarray(bf) != 0.0)),
    )
    return in_maps, flag_key


def kernel(hidden_states, Wq, bq, Wk, bk, Wv, bv, Wo, bo, ln1_g, ln1_b,
           Wi, bi, Wf, bf, ln2_g, ln2_b):
    global LAST_RESULTS
    B, S, _ = hidden_states.shape
    in_maps, flag_key = _prep_inputs(hidden_states, Wo, bo, ln1_g, ln1_b,
                                     Wi, bi, Wf, bf, ln2_g, ln2_b)
    nc = _build(RUN_KWARGS.get("n_reps", 1), flag_key)
    res = run_bass_kernel_spmd(nc, in_maps, list(range(NCORE)),
                               **{k: v for k, v in RUN_KWARGS.items()
                                  if k != "n_reps"})
    LAST_RESULTS = res
    out = np.concatenate([res.results[c]["y"] for c in range(NCORE)], axis=0)
    return np.ascontiguousarray(out.reshape(B, S, D).astype(np.float32))
